# revision 2
# baseline (speedup 1.0000x reference)
"""ConformerBlock Trainium2 Bass kernel — fp8e4m3 DoubleRow edition.

Sharding: data-parallel over batch (B=8) across the 8 NeuronCores; all
weights replicated per core; no collectives.

Per-core layout: feature-major residual stream xT [D=512, S=1024] in SBUF
as a [128, 4, 1024] f32 tile.  All heavy matmuls run in fp8e4m3 with
MatmulPerfMode.DoubleRow (two K-subtiles of 128 per instruction at 0.5
cycles/row), quartering PE row time vs the fp32r baseline.  LayerNorm
statistics stay in fp32r via the all-ones-matmul broadcast trick; attention
scores run in bf16 (K=64); softmax denominators ride an extra ones column
through the fp8 AV matmul and are applied via a K=1 broadcast matmul.

Quantization plan (validated in numpy: end-to-end rel err ~6e-3 < 2e-2):
  - weights pre-scaled host-side by a power of two to fill the e4m3 range;
    descales folded into activation-function input scales / STT scalars.
  - LN outputs h, GLU outputs, conv activations, exp(scores), V and the
    attention context all quantized to e4m3 on the fly (output dtype of the
    existing elementwise ops — no extra instructions).
  - residual stream, LN stats, softmax normalization stay f32.

Engine placement: PE matmuls; Act silu/sigmoid/exp/ln + PSUM->SBUF copies;
DVE PSUM-reading fused multiply-adds (GLU, residual adds, LN stat chain);
Pool (gpsimd) SBUF-only work (squares, LN apply, shifted copies) since
GPSIMD cannot access PSUM.

The depthwise grouped conv (31 taps, groups of 8) is lowered to 16
DoubleRow tap-pair matmuls per (chunk, token-half) against a twice-stored
GLU activation buffer (second copy shifted by one token so each DR
instruction covers taps 2p and 2p+1); tap 31 pairs with a zero matrix.
"""
import sys

sys.path.insert(0, '/opt/trn_rl_repo')

import numpy as np
import ml_dtypes

import concourse.tile as tile
from concourse import bacc, mybir

F32 = mybir.dt.float32
F32R = mybir.dt.float32r
F8 = mybir.dt.float8e4
BF16 = mybir.dt.bfloat16
AF = mybir.ActivationFunctionType
ALU = mybir.AluOpType
DR = mybir.MatmulPerfMode.DoubleRow
E4 = ml_dtypes.float8_e4m3

D = 512            # model dim
S = 1024           # sequence length
B = 8              # batch (one element per core)
HEADS = 8
DH = 64            # head dim
FF_HID = 2048      # ffn hidden (per GLU half)
CONV_IN = 1024     # conv inner dim (per GLU half)
KER = 31
PAD = 15
LN_EPS = 1e-5
BN_EPS = 1e-5
TC = 2             # token chunks
TN = 512           # tokens per chunk
FC = 4             # feature chunks of D
UL = PAD + S + PAD + 2   # padded u length (1056, even for clean strides)

GU = 8.0           # ffn GLU fp8 upscale
UU = 4.0           # conv GLU fp8 upscale
OU = 32.0          # attention context fp8 upscale
EU = 16.0          # exp fp8 upscale

_CACHE = {}


# --------------------------------------------------------------------------
# host-side weight preparation
# --------------------------------------------------------------------------

def _pow2scale(w):
    m = np.abs(w).max()
    if m == 0:
        return 1.0
    return float(2.0 ** np.floor(np.log2(224.0 / m)))


def _q8(w, s):
    return (np.asarray(w, np.float32) * s).astype(E4)


def _prep(inputs):
    g = {k: np.asarray(v, dtype=np.float64) for k, v in inputs.items()}
    p = {}
    sc = {}

    def lin(w, ln_g, ln_b, bias):
        return w * ln_g[None, :], w @ ln_b + bias

    for tag in ("ff1", "ff2"):
        w_in, b_in = lin(g[f"{tag}_win"], g[f"{tag}_g"], g[f"{tag}_b"],
                         g[f"{tag}_bin"])
        s = _pow2scale(w_in)
        sc[f"{tag}_win"] = s
        p[f"{tag}_win_t"] = np.ascontiguousarray(_q8(w_in.T, s))        # [512, 4096]
        p[f"{tag}_bin"] = b_in.astype(np.float32)
        w_out = 0.5 * g[f"{tag}_wout"]
        s = _pow2scale(w_out)
        sc[f"{tag}_wout"] = s
        p[f"{tag}_wout_t"] = np.ascontiguousarray(_q8(w_out.T, s))      # [2048, 512]
        p[f"{tag}_bout"] = (0.5 * g[f"{tag}_bout"]).astype(np.float32)

    wqkv, bqkv = lin(g["wqkv"], g["attn_g"], g["attn_b"], g["bqkv"])
    s = _pow2scale(wqkv)
    sc["wqkv"] = s
    p["wqkv_t"] = np.ascontiguousarray(_q8(wqkv.T, s))                  # [512, 1536]
    p["bqkv"] = bqkv.astype(np.float32)
    s = _pow2scale(g["wo"])
    sc["wo"] = s
    p["wo_t"] = np.ascontiguousarray(_q8(g["wo"].T, s))                 # [512, 512]
    p["bo"] = g["bo"].astype(np.float32)

    pw1, pw1b = lin(g["pw1_w"][:, :, 0], g["conv_g"], g["conv_b"], g["pw1_b"])
    s = _pow2scale(pw1)
    sc["pw1"] = s
    p["pw1_t"] = np.ascontiguousarray(_q8(pw1.T, s))                    # [512, 2048]
    p["pw1_b"] = pw1b.astype(np.float32)

    # depthwise conv + BN fold -> DR tap-pair block-diag mats
    scale = g["bn_g"] / np.sqrt(g["bn_v"] + BN_EPS)                     # [1024]
    dw = g["dw_w"] * scale[:, None, None]                               # [1024, 8, 31]
    dwb = (g["dw_b"] - g["bn_m"]) * scale + g["bn_b"]                   # [1024]
    s = _pow2scale(dw)
    sc["dw"] = s
    dwm = np.zeros((8, 128, 16, 2, 128), np.float64)  # [cc, ki, pair, sub, o]
    o = np.arange(128)
    grp = (o // 8) * 8
    for j in range(8):
        ki = grp + j
        for cc in range(8):
            for tap in range(KER):
                # dwm[cc, ki[o], tap//2, tap%2, o] = dw[cc*128+o, j, tap]
                dwm[cc, ki, tap // 2, tap % 2, o] = dw[cc * 128 + o, j, tap]
    p["dwm"] = np.ascontiguousarray(_q8(dwm, s))
    p["dw_b"] = dwb.astype(np.float32)

    s = _pow2scale(g["pw2_w"])
    sc["pw2"] = s
    p["pw2_t"] = np.ascontiguousarray(_q8(g["pw2_w"][:, :, 0].T, s))    # [1024, 512]
    p["pw2_b"] = g["pw2_b"].astype(np.float32)

    p["fn_g"] = g["fn_g"].astype(np.float32)
    p["fn_b"] = g["fn_b"].astype(np.float32)

    p["ones"] = np.ones((128, 128), np.float32)
    return p, sc


# --------------------------------------------------------------------------
# device program
# --------------------------------------------------------------------------

def _build(cfg, debug=False, nreps=1, phases=("ff1", "attn", "conv", "ff2")):
    sc = dict(cfg["scales"])
    bias_nz = dict(cfg["bias_nz"])
    nc = bacc.Bacc("TRN2", target_bir_lowering=False, debug=False)

    d = {}
    d["x"] = nc.dram_tensor("x", [D, S], F32R, kind="ExternalInput").ap()
    d["ones"] = nc.dram_tensor("ones", [128, 128], F32R, kind="ExternalInput").ap()
    for tag in ("ff1", "ff2"):
        d[f"{tag}_win_t"] = nc.dram_tensor(f"{tag}_win_t", [D, 2 * FF_HID], F8,
                                           kind="ExternalInput").ap()
        d[f"{tag}_wout_t"] = nc.dram_tensor(f"{tag}_wout_t", [FF_HID, D], F8,
                                            kind="ExternalInput").ap()
        d[f"{tag}_bin"] = nc.dram_tensor(f"{tag}_bin", [2 * FF_HID], F32,
                                         kind="ExternalInput").ap()
        d[f"{tag}_bout"] = nc.dram_tensor(f"{tag}_bout", [D], F32,
                                          kind="ExternalInput").ap()
    d["wqkv_t"] = nc.dram_tensor("wqkv_t", [D, 3 * D], F8, kind="ExternalInput").ap()
    d["bqkv"] = nc.dram_tensor("bqkv", [3 * D], F32, kind="ExternalInput").ap()
    d["wo_t"] = nc.dram_tensor("wo_t", [D, D], F8, kind="ExternalInput").ap()
    d["bo"] = nc.dram_tensor("bo", [D], F32, kind="ExternalInput").ap()
    d["pw1_t"] = nc.dram_tensor("pw1_t", [D, 2 * CONV_IN], F8,
                                kind="ExternalInput").ap()
    d["pw1_b"] = nc.dram_tensor("pw1_b", [2 * CONV_IN], F32, kind="ExternalInput").ap()
    d["dwm"] = nc.dram_tensor("dwm", [8, 128, 16 * 2 * 128], F8,
                              kind="ExternalInput").ap()
    d["dw_b"] = nc.dram_tensor("dw_b", [CONV_IN], F32, kind="ExternalInput").ap()
    d["pw2_t"] = nc.dram_tensor("pw2_t", [CONV_IN, D], F8, kind="ExternalInput").ap()
    d["pw2_b"] = nc.dram_tensor("pw2_b", [D], F32, kind="ExternalInput").ap()
    d["fn_g"] = nc.dram_tensor("fn_g", [D], F32, kind="ExternalInput").ap()
    d["fn_b"] = nc.dram_tensor("fn_b", [D], F32, kind="ExternalInput").ap()
    d["out"] = nc.dram_tensor("out", [D, S], F32, kind="ExternalOutput").ap()
    if debug:
        for i in range(1, 5):
            d[f"dbg{i}"] = nc.dram_tensor(f"dbg{i}", [D, S], F32,
                                          kind="ExternalOutput").ap()
        d["dbgh"] = nc.dram_tensor("dbgh", [D, S], F32, kind="ExternalOutput").ap()

    from contextlib import ExitStack
    with tile.TileContext(nc) as tc, ExitStack() as ctx:
        cpool = ctx.enter_context(tc.tile_pool(name="cpool", bufs=1))
        spool = ctx.enter_context(tc.tile_pool(name="spool", bufs=1))
        bp = ctx.enter_context(tc.tile_pool(name="bp", bufs=1))
        ps_acc = ctx.enter_context(tc.tile_pool(name="ps_acc", bufs=4, space="PSUM"))
        ps_s = ctx.enter_context(tc.tile_pool(name="ps_s", bufs=4, space="PSUM"))

        ones = cpool.tile([128, 128], F32R)
        nc.sync.dma_start(ones[:], d["ones"])

        consts = cpool.tile([128, 2], F32, tag="consts")
        nc.gpsimd.memset(consts[:, 0:1], LN_EPS)
        nc.gpsimd.memset(consts[:, 1:2], float(np.log(EU)))
        eps_ap = consts[:, 0:1]
        lneu_ap = consts[:, 1:2]

        xs = spool.tile([128, FC, S], F32R)
        nc.sync.dma_start(xs[:], d["x"].rearrange("(c p) n -> p c n", p=128))

        # per-partition bias rows [128, nchunks] (only loaded when nonzero)
        brow = {}
        for name, width in (("ff1_bin", 2 * FF_HID), ("ff1_bout", D),
                            ("ff2_bin", 2 * FF_HID), ("ff2_bout", D),
                            ("bqkv", 3 * D), ("bo", D), ("pw1_b", 2 * CONV_IN),
                            ("dw_b", CONV_IN), ("pw2_b", D)):
            if bias_nz.get(name):
                t = bp.tile([128, width // 128], F32, tag=name)
                nc.sync.dma_start(t[:], d[name].rearrange("(c p) -> p c", p=128))
                brow[name] = t

        def bias_ap(name, chunk):
            return brow[name][:, chunk:chunk + 1]

        # ------------------------------------------------------------------
        def layer_norm(lnp, t, h8=None, hf=None):
            """LN stats for token chunk t; writes x-hat into h8 (fp8) or hf
            (f32).  Stats via fp32r ones-matmuls (results broadcast across
            partitions); apply via Pool-engine SBUF ops."""
            sl = slice(t * TN, (t + 1) * TN)
            bc_s = ps_s.tile([128, TN], F32, tag="s")
            for c in range(FC):
                nc.tensor.matmul(bc_s[:], ones[:], xs[:, c, sl],
                                 start=(c == 0), stop=(c == FC - 1))
            xsq = lnp.tile([128, FC, TN], F32R, tag="xsq")
            for c in range(FC):
                nc.gpsimd.tensor_tensor(xsq[:, c, :], xs[:, c, sl].bitcast(F32),
                                        xs[:, c, sl].bitcast(F32), ALU.mult)
            bc_q = ps_s.tile([128, TN], F32, tag="s")
            for c in range(FC):
                nc.tensor.matmul(bc_q[:], ones[:], xsq[:, c, :],
                                 start=(c == 0), stop=(c == FC - 1))
            mu = lnp.tile([128, TN], F32, tag="mu")
            nc.vector.tensor_scalar(mu[:], bc_s[:], 1.0 / D, None, ALU.mult)
            m2 = lnp.tile([128, TN], F32, tag="m2")
            nc.gpsimd.tensor_tensor(m2[:], mu[:], mu[:], ALU.mult)
            # t1 = bc_q - mu^2 * D
            t1 = lnp.tile([128, TN], F32, tag="t1")
            nc.vector.scalar_tensor_tensor(t1[:], m2[:], -float(D), bc_q[:],
                                           ALU.mult, ALU.add)
            # rsig = exp(-0.5 * ln(t1/D + eps)) = rsqrt(var + eps)
            lnt = lnp.tile([128, TN], F32, tag="lnt")
            nc.scalar.activation(lnt[:], t1[:], AF.Ln, scale=1.0 / D, bias=eps_ap)
            rsig = lnp.tile([128, TN], F32, tag="rsig")
            nc.scalar.activation(rsig[:], lnt[:], AF.Exp, scale=-0.5)
            # Bt = mu * rsig  (per-token offset, broadcast over partitions)
            bt = lnp.tile([128, TN], F32, tag="bt")
            nc.gpsimd.tensor_tensor(bt[:], mu[:], rsig[:], ALU.mult)
            for c in range(FC):
                xr = lnp.tile([128, TN], F32, tag="xr")
                nc.gpsimd.tensor_tensor(xr[:], xs[:, c, sl].bitcast(F32),
                                        rsig[:], ALU.mult)
                dst = h8[:, c, sl] if h8 is not None else hf[:, c, sl]
                nc.gpsimd.tensor_tensor(dst, xr[:], bt[:], ALU.subtract)

        # ------------------------------------------------------------------
        def ffn(tag, pools, dbg=False):
            lnp, hpool, wff, fsp = pools
            h = hpool.tile([128, FC, S], F8, tag="h")
            for t in range(TC):
                layer_norm(lnp, t, h8=h)
            if dbg:
                hdbg = hpool.tile([128, FC, S], F32, tag="hdbg")
                for c in range(FC):
                    nc.vector.tensor_scalar(hdbg[:, c, :], h[:, c, :], 1.0,
                                            None, ALU.mult)
                nc.sync.dma_start(d["dbgh"].rearrange("(c p) n -> p c n", p=128),
                                  hdbg[:])
            w_in = wff.tile([128, FC, 2 * FF_HID], F8, tag="wffin")
            nc.sync.dma_start(w_in[:],
                              d[f"{tag}_win_t"].rearrange("(c p) m -> p c m", p=128))
            nh = FF_HID // 128  # 16
            w_out = wff.tile([128, nh, D], F8, tag="wffout")
            nc.sync.dma_start(w_out[:],
                              d[f"{tag}_wout_t"].rearrange("(c p) m -> p c m", p=128))
            b_in = f"{tag}_bin"
            b_out = f"{tag}_bout"
            s_in = 1.0 / sc[f"{tag}_win"]
            s_g = GU / sc[f"{tag}_win"]
            s_out = 1.0 / (GU * sc[f"{tag}_wout"])
            for t in range(TC):
                sl = slice(t * TN, (t + 1) * TN)
                accs = [ps_acc.tile([128, TN], F32, tag="acc", name="acc")
                        for _ in range(FC)]
                for hp in range(nh // 2):        # hc pairs
                    g2 = fsp.tile([128, 2, TN], F8, tag="g2")
                    for sub in range(2):
                        hc = 2 * hp + sub
                        a_ps = ps_s.tile([128, TN], F32, tag="s")
                        nc.tensor.matmul(a_ps[:], w_in[:, 0:2, hc * 128:(hc + 1) * 128],
                                         h[:, 0:2, sl], start=True, stop=False,
                                         perf_mode=DR)
                        nc.tensor.matmul(a_ps[:], w_in[:, 2:4, hc * 128:(hc + 1) * 128],
                                         h[:, 2:4, sl], start=False, stop=True,
                                         perf_mode=DR)
                        mo = nh + hc
                        c_ps = ps_s.tile([128, TN], F32, tag="s")
                        nc.tensor.matmul(c_ps[:], w_in[:, 0:2, mo * 128:(mo + 1) * 128],
                                         h[:, 0:2, sl], start=True, stop=False,
                                         perf_mode=DR)
                        nc.tensor.matmul(c_ps[:], w_in[:, 2:4, mo * 128:(mo + 1) * 128],
                                         h[:, 2:4, sl], start=False, stop=True,
                                         perf_mode=DR)
                        a_sb = fsp.tile([128, TN], F32, tag="asb")
                        nc.scalar.activation(
                            a_sb[:], a_ps[:], AF.Silu, scale=s_in,
                            bias=bias_ap(b_in, hc) if bias_nz[b_in] else 0.0)
                        if bias_nz[b_in]:
                            csb = fsp.tile([128, TN], F32, tag="csb")
                            nc.scalar.activation(csb[:], c_ps[:], AF.Identity,
                                                 scale=s_g, bias=bias_ap(b_in, mo))
                            nc.vector.tensor_tensor(g2[:, sub, :], csb[:],
                                                    a_sb[:], ALU.mult)
                        else:
                            nc.vector.scalar_tensor_tensor(
                                g2[:, sub, :], c_ps[:], s_g, a_sb[:],
                                ALU.mult, ALU.mult)
                    for m in range(FC):
                        nc.tensor.matmul(accs[m][:],
                                         w_out[:, 2 * hp:2 * hp + 2,
                                               m * 128:(m + 1) * 128],
                                         g2[:, 0:2, :],
                                         start=(hp == 0), stop=(hp == nh // 2 - 1),
                                         perf_mode=DR)
                for m in range(FC):
                    if bias_nz[b_out]:
                        asb = fsp.tile([128, TN], F32, tag="rsb")
                        nc.scalar.activation(asb[:], accs[m][:], AF.Identity,
                                             scale=s_out, bias=bias_ap(b_out, m))
                        nc.vector.tensor_tensor(xs[:, m, sl], asb[:],
                                                xs[:, m, sl].bitcast(F32), ALU.add)
                    else:
                        nc.vector.scalar_tensor_tensor(
                            xs[:, m, sl], accs[m][:], s_out,
                            xs[:, m, sl].bitcast(F32), ALU.mult, ALU.add)

        # ------------------------------------------------------------------
        def attention(pools):
            lnp, hpool, watt, attp, att2, att3 = pools
            h = hpool.tile([128, FC, S], F8, tag="h")
            for t in range(TC):
                layer_norm(lnp, t, h8=h)
            wqkv = watt.tile([128, FC, 3 * D], F8, tag="wqkv")
            nc.sync.dma_start(wqkv[:],
                              d["wqkv_t"].rearrange("(c p) m -> p c m", p=128))
            wo = watt.tile([128, FC, D], F8, tag="wo")
            nc.sync.dma_start(wo[:], d["wo_t"].rearrange("(c p) m -> p c m", p=128))
            s_qkv = 1.0 / sc["wqkv"]
            bvq = None
            if bias_nz["bqkv"]:
                bvq = bp.tile([128, FC], F32, tag="bvq")
                nc.sync.dma_start(bvq[:],
                                  d["bqkv"][2 * D:3 * D].rearrange("(c p) -> p c",
                                                                   p=128))
                # o_fm carries an OU upscale; pre-scale the v bias to match
                nc.vector.tensor_scalar(bvq[:], bvq[:], OU, None, ALU.mult)

            q_sb = attp.tile([128, FC, S], BF16, tag="q")
            k_sb = attp.tile([128, FC, S], BF16, tag="k")
            for fc in range(FC):
                for t in range(TC):
                    sl = slice(t * TN, (t + 1) * TN)
                    for which, base in (("q", 0), ("k", D)):
                        pp = ps_s.tile([128, TN], F32, tag="s")
                        mo = base // 128 + fc
                        nc.tensor.matmul(pp[:],
                                         wqkv[:, 0:2, mo * 128:(mo + 1) * 128],
                                         h[:, 0:2, sl], start=True, stop=False,
                                         perf_mode=DR)
                        nc.tensor.matmul(pp[:],
                                         wqkv[:, 2:4, mo * 128:(mo + 1) * 128],
                                         h[:, 2:4, sl], start=False, stop=True,
                                         perf_mode=DR)
                        dst = q_sb if which == "q" else k_sb
                        nc.scalar.activation(
                            dst[:, fc, sl], pp[:], AF.Copy, scale=s_qkv,
                            bias=0.0)
                        if bias_nz["bqkv"]:
                            nc.vector.tensor_scalar(
                                dst[:, fc, sl], dst[:, fc, sl].bitcast(BF16),
                                bias_ap("bqkv", mo), None, ALU.add)

            # v token-major with fp8 ones column at index 64 per head
            vaug = attp.tile([128, 8, HEADS, 66], F8, tag="vaug")
            nc.gpsimd.memset(vaug[:, :, :, 64:65], 1.0)
            for kc in range(8):
                v_ps = ps_s.tile([128, D], F32, tag="s")
                nc.tensor.matmul(v_ps[:], h[:, 0:2, kc * 128:(kc + 1) * 128],
                                 wqkv[:, 0:2, 2 * D:3 * D], start=True, stop=False,
                                 perf_mode=DR)
                nc.tensor.matmul(v_ps[:], h[:, 2:4, kc * 128:(kc + 1) * 128],
                                 wqkv[:, 2:4, 2 * D:3 * D], start=False, stop=True,
                                 perf_mode=DR)
                nc.scalar.activation(
                    vaug[:, kc, :, 0:64],
                    v_ps[:].rearrange("p (h e) -> p h e", h=HEADS), AF.Copy,
                    scale=s_qkv)

            o_fm = attp.tile([128, FC, S], F8, tag="ofm")
            sm_scale = float(DH) ** -0.5
            for hd in range(HEADS):
                hb = (hd % 2) * 64
                hc = hd // 2
                for t in range(TC):
                    sl = slice(t * TN, (t + 1) * TN)
                    e_sb = att2.tile([128, 8, TN], F8, tag="esb")
                    o_ps = ps_acc.tile([65, TN], F32, tag="acc")
                    for kc in range(8):
                        s_ps = ps_s.tile([128, TN], F32, tag="s")
                        nc.tensor.matmul(s_ps[:],
                                         k_sb[hb:hb + 64, hc, kc * 128:(kc + 1) * 128],
                                         q_sb[hb:hb + 64, hc, sl],
                                         start=True, stop=True)
                        nc.scalar.activation(e_sb[:, kc, :], s_ps[:], AF.Exp,
                                             scale=sm_scale, bias=lneu_ap)
                    for kp in range(4):
                        nc.tensor.matmul(o_ps[:],
                                         vaug[:, 2 * kp:2 * kp + 2, hd, 0:65],
                                         e_sb[:, 2 * kp:2 * kp + 2, :],
                                         start=(kp == 0), stop=(kp == 3),
                                         perf_mode=DR)
                    rows = att3.tile([1, 3, TN], F32, tag="rows")
                    nc.scalar.activation(rows[:, 2, :], o_ps[64:65, :], AF.Copy)
                    nc.vector.reciprocal_approx_accurate(
                        rows[:, 0, :], rows[:, 2, :], rows[:, 1, :])
                    rrow_r = att3.tile([1, TN], F32R, tag="rrowr")
                    nc.scalar.activation(rrow_r[:], rows[:, 0, :], AF.Copy,
                                         scale=OU)
                    bc_ps = ps_s.tile([64, TN], F32, tag="s")
                    nc.tensor.matmul(bc_ps[:], ones[0:1, 0:64], rrow_r[:],
                                     start=True, stop=True)
                    bc_sb = att3.tile([64, TN], F32, tag="bcsb")
                    nc.scalar.activation(bc_sb[:], bc_ps[:], AF.Copy)
                    nc.vector.tensor_tensor(o_fm[hb:hb + 64, hc, sl],
                                            o_ps[0:64, :], bc_sb[:], ALU.mult)
                    if bias_nz["bqkv"]:
                        # + v bias (softmax weights sum to one; bvq pre-scaled)
                        nc.vector.tensor_scalar(
                            o_fm[hb:hb + 64, hc, sl],
                            o_fm[hb:hb + 64, hc, sl].bitcast(F8),
                            bvq[hb:hb + 64, hc:hc + 1], None, ALU.add)

            s_o = 1.0 / (OU * sc["wo"])
            for t in range(TC):
                sl = slice(t * TN, (t + 1) * TN)
                for m in range(FC):
                    acc = ps_acc.tile([128, TN], F32, tag="acc")
                    nc.tensor.matmul(acc[:], wo[:, 0:2, m * 128:(m + 1) * 128],
                                     o_fm[:, 0:2, sl], start=True, stop=False,
                                     perf_mode=DR)
                    nc.tensor.matmul(acc[:], wo[:, 2:4, m * 128:(m + 1) * 128],
                                     o_fm[:, 2:4, sl], start=False, stop=True,
                                     perf_mode=DR)
                    if bias_nz["bo"]:
                        tmp = att3.tile([128, TN], F32, tag="tmp")
                        nc.scalar.activation(tmp[:], acc[:], AF.Identity,
                                             scale=s_o, bias=bias_ap("bo", m))
                        nc.vector.tensor_tensor(xs[:, m, sl], tmp[:],
                                                xs[:, m, sl].bitcast(F32), ALU.add)
                    else:
                        nc.vector.scalar_tensor_tensor(
                            xs[:, m, sl], acc[:], s_o,
                            xs[:, m, sl].bitcast(F32), ALU.mult, ALU.add)

        # ------------------------------------------------------------------
        def conv(pools):
            lnp, hpool, wconv, wdw, convp, up, fsp = pools
            h = hpool.tile([128, FC, S], F8, tag="h")
            for t in range(TC):
                layer_norm(lnp, t, h8=h)
            pw1 = wconv.tile([128, FC, 2 * CONV_IN], F8, tag="pw1")
            nc.sync.dma_start(pw1[:],
                              d["pw1_t"].rearrange("(c p) m -> p c m", p=128))
            ncc = CONV_IN // 128  # 8
            pw2 = wconv.tile([128, ncc, D], F8, tag="pw2")
            nc.sync.dma_start(pw2[:],
                              d["pw2_t"].rearrange("(c p) m -> p c m", p=128))
            s_p1 = 1.0 / sc["pw1"]
            s_u = UU / sc["pw1"]
            s_dw = 1.0 / (UU * sc["dw"])
            s_p2 = 1.0 / sc["pw2"]

            dvo = convp.tile([128, ncc, S], F8, tag="dvo")
            for cc in range(ncc):
                u2 = up.tile([128, 2, UL], F8, tag="u")
                nc.gpsimd.memset(u2[:, :, 0:PAD], 0.0)
                nc.gpsimd.memset(u2[:, :, PAD + S:], 0.0)
                dwW = wdw.tile([128, 16, 2, 128], F8, tag="dww")
                nc.sync.dma_start(
                    dwW[:], d["dwm"][cc].rearrange("p (t s m) -> p t s m",
                                                   t=16, s=2))
                for t in range(TC):
                    sl = slice(t * TN, (t + 1) * TN)
                    a_ps = ps_s.tile([128, TN], F32, tag="s")
                    nc.tensor.matmul(a_ps[:], pw1[:, 0:2, cc * 128:(cc + 1) * 128],
                                     h[:, 0:2, sl], start=True, stop=False,
                                     perf_mode=DR)
                    nc.tensor.matmul(a_ps[:], pw1[:, 2:4, cc * 128:(cc + 1) * 128],
                                     h[:, 2:4, sl], start=False, stop=True,
                                     perf_mode=DR)
                    mo = ncc + cc
                    c_ps = ps_s.tile([128, TN], F32, tag="s")
                    nc.tensor.matmul(c_ps[:], pw1[:, 0:2, mo * 128:(mo + 1) * 128],
                                     h[:, 0:2, sl], start=True, stop=False,
                                     perf_mode=DR)
                    nc.tensor.matmul(c_ps[:], pw1[:, 2:4, mo * 128:(mo + 1) * 128],
                                     h[:, 2:4, sl], start=False, stop=True,
                                     perf_mode=DR)
                    sg = fsp.tile([128, TN], F32, tag="sg")
                    nc.scalar.activation(
                        sg[:], c_ps[:], AF.Sigmoid, scale=s_p1,
                        bias=bias_ap("pw1_b", mo) if bias_nz["pw1_b"] else 0.0)
                    if bias_nz["pw1_b"]:
                        asb = fsp.tile([128, TN], F32, tag="asb")
                        nc.scalar.activation(asb[:], a_ps[:], AF.Identity,
                                             scale=s_u, bias=bias_ap("pw1_b", cc))
                        nc.vector.tensor_tensor(
                            u2[:, 0, PAD + t * TN:PAD + (t + 1) * TN],
                            asb[:], sg[:], ALU.mult)
                    else:
                        nc.vector.scalar_tensor_tensor(
                            u2[:, 0, PAD + t * TN:PAD + (t + 1) * TN],
                            a_ps[:], s_u, sg[:], ALU.mult, ALU.mult)
                # shifted copy for odd taps: u2[:,1,i] = u2[:,0,i+1]
                nc.gpsimd.tensor_scalar(u2[:, 1, PAD - 1:PAD + S],
                                        u2[:, 0, PAD:PAD + S + 1], 1.0, None,
                                        ALU.mult)
                for t in range(TC):
                    acc = ps_s.tile([128, TN], F32, tag="s")
                    for pr in range(16):
                        nc.tensor.matmul(acc[:], dwW[:, pr, :, :],
                                         u2[:, 0:2, t * TN + 2 * pr:
                                            t * TN + 2 * pr + TN],
                                         start=(pr == 0), stop=(pr == 15),
                                         perf_mode=DR)
                    nc.scalar.activation(
                        dvo[:, cc, t * TN:(t + 1) * TN], acc[:], AF.Silu,
                        scale=s_dw,
                        bias=bias_ap("dw_b", cc) if bias_nz["dw_b"] else 0.0)

            for t in range(TC):
                sl = slice(t * TN, (t + 1) * TN)
                for m in range(FC):
                    acc = ps_acc.tile([128, TN], F32, tag="acc")
                    for cp in range(4):
                        nc.tensor.matmul(acc[:],
                                         pw2[:, 2 * cp:2 * cp + 2,
                                             m * 128:(m + 1) * 128],
                                         dvo[:, 2 * cp:2 * cp + 2, sl],
                                         start=(cp == 0), stop=(cp == 3),
                                         perf_mode=DR)
                    if bias_nz["pw2_b"]:
                        tmp = fsp.tile([128, TN], F32, tag="tmp")
                        nc.scalar.activation(tmp[:], acc[:], AF.Identity,
                                             scale=s_p2, bias=bias_ap("pw2_b", m))
                        nc.vector.tensor_tensor(xs[:, m, sl], tmp[:],
                                                xs[:, m, sl].bitcast(F32), ALU.add)
                    else:
                        nc.vector.scalar_tensor_tensor(
                            xs[:, m, sl], acc[:], s_p2,
                            xs[:, m, sl].bitcast(F32), ALU.mult, ALU.add)

        # ------------------------------------------------------------------
        def ff_pools(st):
            return (st.enter_context(tc.tile_pool(name="lnp", bufs=2)),
                    st.enter_context(tc.tile_pool(name="hp", bufs=1)),
                    st.enter_context(tc.tile_pool(name="wff", bufs=1)),
                    st.enter_context(tc.tile_pool(name="fsp", bufs=3)))

        def dbg_tap(i):
            if debug:
                nc.sync.dma_start(d[f"dbg{i}"].rearrange("(c p) n -> p c n", p=128),
                                  xs[:].bitcast(F32))

        for _rep in range(nreps):
            dbg = debug and _rep == nreps - 1
            if "ff1" in phases:
                with ExitStack() as st:
                    ffn("ff1", ff_pools(st), dbg=dbg)
            if dbg:
                dbg_tap(1)
            if "attn" in phases:
                with ExitStack() as st:
                    pools = (st.enter_context(tc.tile_pool(name="lnp", bufs=2)),
                             st.enter_context(tc.tile_pool(name="hp", bufs=1)),
                             st.enter_context(tc.tile_pool(name="watt", bufs=1)),
                             st.enter_context(tc.tile_pool(name="attp", bufs=1)),
                             st.enter_context(tc.tile_pool(name="att2", bufs=2)),
                             st.enter_context(tc.tile_pool(name="att3", bufs=2)))
                    attention(pools)
            if dbg:
                dbg_tap(2)
            if "conv" in phases:
                with ExitStack() as st:
                    pools = (st.enter_context(tc.tile_pool(name="lnp", bufs=2)),
                             st.enter_context(tc.tile_pool(name="hp", bufs=1)),
                             st.enter_context(tc.tile_pool(name="wconv", bufs=1)),
                             st.enter_context(tc.tile_pool(name="wdw", bufs=2)),
                             st.enter_context(tc.tile_pool(name="convp", bufs=1)),
                             st.enter_context(tc.tile_pool(name="up", bufs=3)),
                             st.enter_context(tc.tile_pool(name="fsp", bufs=2)))
                    conv(pools)
            if dbg:
                dbg_tap(3)
            if "ff2" in phases:
                with ExitStack() as st:
                    ffn("ff2", ff_pools(st))
            if dbg:
                dbg_tap(4)

        with ExitStack() as st:
            lnp = st.enter_context(tc.tile_pool(name="lnp", bufs=2))
            outt = spool.tile([128, FC, S], F32, tag="outt")
            for t in range(TC):
                layer_norm(lnp, t, hf=outt)
            if bias_nz["fn"]:
                fg = cpool.tile([128, FC], F32, tag="fg")
                nc.sync.dma_start(fg[:], d["fn_g"].rearrange("(c p) -> p c", p=128))
                fb = cpool.tile([128, FC], F32, tag="fb")
                nc.sync.dma_start(fb[:], d["fn_b"].rearrange("(c p) -> p c", p=128))
                for c in range(FC):
                    nc.vector.tensor_scalar(outt[:, c, :], outt[:, c, :],
                                            fg[:, c:c + 1], fb[:, c:c + 1],
                                            ALU.mult, ALU.add)
        nc.sync.dma_start(d["out"].rearrange("(c p) n -> p c n", p=128), outt[:])

    nc.compile()
    return nc


# --------------------------------------------------------------------------
# SPMD execution (replicates bass2jax.run_bass_via_pjrt, reusable executable)
# --------------------------------------------------------------------------

class _Runner:
    def __init__(self, nc, n_cores=8):
        import jax
        from jax.sharding import Mesh, PartitionSpec
        from jax.experimental.shard_map import shard_map
        from concourse.bass2jax import (
            _bass_exec_p, install_neuronx_cc_hook, partition_id_tensor,
        )
        install_neuronx_cc_hook()
        self.jax = jax
        self.n_cores = n_cores
        partition_name = (nc.partition_id_tensor.name
                          if nc.partition_id_tensor else None)
        in_names, out_names, out_avals, zero_outs = [], [], [], []
        for alloc in nc.m.functions[0].allocations:
            if not isinstance(alloc, mybir.MemoryLocationSet):
                continue
            name = alloc.memorylocations[0].name
            if alloc.kind == "ExternalInput":
                if name != partition_name:
                    in_names.append(name)
            elif alloc.kind == "ExternalOutput":
                shape = tuple(alloc.tensor_shape)
                dtype = mybir.dt.np(alloc.dtype)
                out_names.append(name)
                out_avals.append(jax.core.ShapedArray(shape, dtype))
                zero_outs.append(np.zeros(shape, dtype))
        self.in_names, self.out_names = in_names, out_names
        self.out_avals, self.zero_outs = out_avals, zero_outs
        n_params, n_outs = len(in_names), len(out_avals)
        all_in = list(in_names) + list(out_names)
        if partition_name is not None:
            all_in.append(partition_name)
        donate = tuple(range(n_params, n_params + n_outs))

        def _body(*args):
            operands = list(args)
            if partition_name is not None:
                operands.append(partition_id_tensor())
            return tuple(_bass_exec_p.bind(
                *operands, out_avals=tuple(out_avals), in_names=tuple(all_in),
                out_names=tuple(out_names), lowering_input_output_aliases=(),
                sim_require_finite=True, sim_require_nnan=True, nc=nc))

        devices = jax.devices()[:n_cores]
        mesh = Mesh(np.asarray(devices), ("core",))
        in_specs = (PartitionSpec("core"),) * (n_params + n_outs)
        out_specs = (PartitionSpec("core"),) * n_outs
        self._fn = jax.jit(
            shard_map(_body, mesh=mesh, in_specs=in_specs, out_specs=out_specs,
                      check_rep=False),
            donate_argnums=donate, keep_unused=True)

    def concat_inputs(self, in_maps):
        n = self.n_cores
        per_core = [[np.asarray(m[name]) for name in self.in_names]
                    for m in in_maps]
        return [np.concatenate([per_core[c][i] for c in range(n)], axis=0)
                for i in range(len(self.in_names))]

    def run_concat(self, concat_in):
        n = self.n_cores
        zeros = [np.zeros((n * z.shape[0], *z.shape[1:]), z.dtype)
                 for z in self.zero_outs]
        out = self._fn(*concat_in, *zeros)
        self.jax.block_until_ready(out)
        return out

    def __call__(self, in_maps):
        out = self.run_concat(self.concat_inputs(in_maps))
        n = self.n_cores
        return [
            {name: np.asarray(out[i]).reshape(n, *self.out_avals[i].shape)[c]
             for i, name in enumerate(self.out_names)}
            for c in range(n)
        ]


def _freeze(obj):
    if isinstance(obj, dict):
        return tuple(sorted((k, _freeze(v)) for k, v in obj.items()))
    return obj


def _get_runner(cfg, debug=False, nreps=1,
                phases=("ff1", "attn", "conv", "ff2")):
    key = (_freeze(cfg), debug, nreps, tuple(phases))
    if key not in _CACHE:
        _CACHE[key] = _Runner(
            _build(cfg, debug=debug, nreps=nreps, phases=phases), 8)
    return _CACHE[key]


def _make_in_maps(inputs):
    p, scales = _prep(inputs)
    x = np.asarray(inputs["x"], np.float32)
    bias_nz = {
        "ff1_bin": bool(np.any(p["ff1_bin"])), "ff1_bout": bool(np.any(p["ff1_bout"])),
        "ff2_bin": bool(np.any(p["ff2_bin"])), "ff2_bout": bool(np.any(p["ff2_bout"])),
        "bqkv": bool(np.any(p["bqkv"])), "bo": bool(np.any(p["bo"])),
        "pw1_b": bool(np.any(p["pw1_b"])), "dw_b": bool(np.any(p["dw_b"])),
        "pw2_b": bool(np.any(p["pw2_b"])),
        "fn": bool(np.any(p["fn_g"] != 1.0) or np.any(p["fn_b"])),
    }
    cfg = {"scales": scales, "bias_nz": bias_nz}
    p["dwm"] = p["dwm"].reshape(8, 128, 16 * 2 * 128)
    shared = {k: p[k] for k in
              ("ones", "ff1_win_t", "ff1_wout_t", "ff1_bin", "ff1_bout",
               "ff2_win_t", "ff2_wout_t", "ff2_bin", "ff2_bout",
               "wqkv_t", "bqkv", "wo_t", "bo", "pw1_t", "pw1_b", "dwm", "dw_b",
               "pw2_t", "pw2_b", "fn_g", "fn_b")}
    in_maps = []
    for b in range(B):
        m = dict(shared)
        m["x"] = np.ascontiguousarray(x[b].T)          # [512, 1024]
        in_maps.append(m)
    return in_maps, cfg


def kernel(**inputs):
    in_maps, cfg = _make_in_maps(inputs)
    runner = _get_runner(cfg)
    results = runner(in_maps)
    out = np.stack([results[b]["out"].T for b in range(B)], axis=0)
    return np.ascontiguousarray(out.astype(np.float32))


# revision 4
# speedup vs baseline: 1.1935x; 1.1935x over previous
"""ConformerBlock Trainium2 Bass kernel — fp8e4m3 DoubleRow edition.

Sharding: data-parallel over batch (B=8) across the 8 NeuronCores; all
weights replicated per core; no collectives.

Per-core layout: feature-major residual stream xT [D=512, S=1024] in SBUF
as a [128, 4, 1024] f32 tile.  All heavy matmuls run in fp8e4m3 with
MatmulPerfMode.DoubleRow (two K-subtiles of 128 per instruction at 0.5
cycles/row), quartering PE row time vs the fp32r baseline.  LayerNorm
statistics stay in fp32r via the all-ones-matmul broadcast trick; attention
scores run in bf16 (K=64); softmax denominators ride an extra ones column
through the fp8 AV matmul and are applied via a K=1 broadcast matmul.

Quantization plan (validated in numpy: end-to-end rel err ~6e-3 < 2e-2):
  - weights pre-scaled host-side by a power of two to fill the e4m3 range;
    descales folded into activation-function input scales / STT scalars.
  - LN outputs h, GLU outputs, conv activations, exp(scores), V and the
    attention context all quantized to e4m3 on the fly (output dtype of the
    existing elementwise ops — no extra instructions).
  - residual stream, LN stats, softmax normalization stay f32.

Engine placement: PE matmuls; Act silu/sigmoid/exp/ln + PSUM->SBUF copies;
DVE PSUM-reading fused multiply-adds (GLU, residual adds, LN stat chain);
Pool (gpsimd) SBUF-only work (squares, LN apply, shifted copies) since
GPSIMD cannot access PSUM.

The depthwise grouped conv (31 taps, groups of 8) is lowered to 16
DoubleRow tap-pair matmuls per (chunk, token-half) against a twice-stored
GLU activation buffer (second copy shifted by one token so each DR
instruction covers taps 2p and 2p+1); tap 31 pairs with a zero matrix.
"""
import sys

sys.path.insert(0, '/opt/trn_rl_repo')

import numpy as np
import ml_dtypes

import concourse.tile as tile
from concourse import bacc, mybir

F32 = mybir.dt.float32
F32R = mybir.dt.float32r
F8 = mybir.dt.float8e4
BF16 = mybir.dt.bfloat16
AF = mybir.ActivationFunctionType
ALU = mybir.AluOpType
DR = mybir.MatmulPerfMode.DoubleRow
E4 = ml_dtypes.float8_e4m3

D = 512            # model dim
S = 1024           # sequence length
B = 8              # batch (one element per core)
HEADS = 8
DH = 64            # head dim
FF_HID = 2048      # ffn hidden (per GLU half)
CONV_IN = 1024     # conv inner dim (per GLU half)
KER = 31
PAD = 15
LN_EPS = 1e-5
BN_EPS = 1e-5
TC = 2             # token chunks
TN = 512           # tokens per chunk
FC = 4             # feature chunks of D
UL = PAD + S + PAD + 2   # padded u length (1056, even for clean strides)

GU = 8.0           # ffn GLU fp8 upscale
UU = 4.0           # conv GLU fp8 upscale
OU = 32.0          # attention context fp8 upscale
EU = 16.0          # exp fp8 upscale

_CACHE = {}


# --------------------------------------------------------------------------
# host-side weight preparation
# --------------------------------------------------------------------------

def _pow2scale(w):
    m = np.abs(w).max()
    if m == 0:
        return 1.0
    return float(2.0 ** np.floor(np.log2(224.0 / m)))


def _q8(w, s):
    return (np.asarray(w, np.float32) * s).astype(E4)


def _prep(inputs):
    g = {k: np.asarray(v, dtype=np.float64) for k, v in inputs.items()}
    p = {}
    sc = {}

    def lin(w, ln_g, ln_b, bias):
        return w * ln_g[None, :], w @ ln_b + bias

    for tag in ("ff1", "ff2"):
        w_in, b_in = lin(g[f"{tag}_win"], g[f"{tag}_g"], g[f"{tag}_b"],
                         g[f"{tag}_bin"])
        s = _pow2scale(w_in)
        sc[f"{tag}_win"] = s
        p[f"{tag}_win_t"] = np.ascontiguousarray(_q8(w_in.T, s))        # [512, 4096]
        p[f"{tag}_bin"] = b_in.astype(np.float32)
        w_out = 0.5 * g[f"{tag}_wout"]
        s = _pow2scale(w_out)
        sc[f"{tag}_wout"] = s
        p[f"{tag}_wout_t"] = np.ascontiguousarray(_q8(w_out.T, s))      # [2048, 512]
        p[f"{tag}_bout"] = (0.5 * g[f"{tag}_bout"]).astype(np.float32)

    wqkv, bqkv = lin(g["wqkv"], g["attn_g"], g["attn_b"], g["bqkv"])
    s = _pow2scale(wqkv)
    sc["wqkv"] = s
    p["wqkv_t"] = np.ascontiguousarray(_q8(wqkv.T, s))                  # [512, 1536]
    p["bqkv"] = bqkv.astype(np.float32)
    s = _pow2scale(g["wo"])
    sc["wo"] = s
    p["wo_t"] = np.ascontiguousarray(_q8(g["wo"].T, s))                 # [512, 512]
    p["bo"] = g["bo"].astype(np.float32)

    pw1, pw1b = lin(g["pw1_w"][:, :, 0], g["conv_g"], g["conv_b"], g["pw1_b"])
    s = _pow2scale(pw1)
    sc["pw1"] = s
    p["pw1_t"] = np.ascontiguousarray(_q8(pw1.T, s))                    # [512, 2048]
    p["pw1_b"] = pw1b.astype(np.float32)

    # depthwise conv + BN fold -> DR tap-pair block-diag mats
    scale = g["bn_g"] / np.sqrt(g["bn_v"] + BN_EPS)                     # [1024]
    dw = g["dw_w"] * scale[:, None, None]                               # [1024, 8, 31]
    dwb = (g["dw_b"] - g["bn_m"]) * scale + g["bn_b"]                   # [1024]
    s = _pow2scale(dw)
    sc["dw"] = s
    dwm = np.zeros((8, 128, 16, 2, 128), np.float64)  # [cc, ki, pair, sub, o]
    o = np.arange(128)
    grp = (o // 8) * 8
    for j in range(8):
        ki = grp + j
        for cc in range(8):
            for tap in range(KER):
                # dwm[cc, ki[o], tap//2, tap%2, o] = dw[cc*128+o, j, tap]
                dwm[cc, ki, tap // 2, tap % 2, o] = dw[cc * 128 + o, j, tap]
    p["dwm"] = np.ascontiguousarray(_q8(dwm, s))
    p["dw_b"] = dwb.astype(np.float32)

    s = _pow2scale(g["pw2_w"])
    sc["pw2"] = s
    p["pw2_t"] = np.ascontiguousarray(_q8(g["pw2_w"][:, :, 0].T, s))    # [1024, 512]
    p["pw2_b"] = g["pw2_b"].astype(np.float32)

    p["fn_g"] = g["fn_g"].astype(np.float32)
    p["fn_b"] = g["fn_b"].astype(np.float32)

    p["ones"] = np.ones((128, 128), np.float32)
    return p, sc


# --------------------------------------------------------------------------
# device program
# --------------------------------------------------------------------------

def _build(cfg, debug=False, nreps=1, phases=("ff1", "attn", "conv", "ff2")):
    sc = dict(cfg["scales"])
    bias_nz = dict(cfg["bias_nz"])
    nc = bacc.Bacc("TRN2", target_bir_lowering=False, debug=False)

    d = {}
    d["x"] = nc.dram_tensor("x", [D, S], F32R, kind="ExternalInput").ap()
    d["ones"] = nc.dram_tensor("ones", [128, 128], F32R, kind="ExternalInput").ap()
    for tag in ("ff1", "ff2"):
        d[f"{tag}_win_t"] = nc.dram_tensor(f"{tag}_win_t", [D, 2 * FF_HID], F8,
                                           kind="ExternalInput").ap()
        d[f"{tag}_wout_t"] = nc.dram_tensor(f"{tag}_wout_t", [FF_HID, D], F8,
                                            kind="ExternalInput").ap()
        d[f"{tag}_bin"] = nc.dram_tensor(f"{tag}_bin", [2 * FF_HID], F32,
                                         kind="ExternalInput").ap()
        d[f"{tag}_bout"] = nc.dram_tensor(f"{tag}_bout", [D], F32,
                                          kind="ExternalInput").ap()
    d["wqkv_t"] = nc.dram_tensor("wqkv_t", [D, 3 * D], F8, kind="ExternalInput").ap()
    d["bqkv"] = nc.dram_tensor("bqkv", [3 * D], F32, kind="ExternalInput").ap()
    d["wo_t"] = nc.dram_tensor("wo_t", [D, D], F8, kind="ExternalInput").ap()
    d["bo"] = nc.dram_tensor("bo", [D], F32, kind="ExternalInput").ap()
    d["pw1_t"] = nc.dram_tensor("pw1_t", [D, 2 * CONV_IN], F8,
                                kind="ExternalInput").ap()
    d["pw1_b"] = nc.dram_tensor("pw1_b", [2 * CONV_IN], F32, kind="ExternalInput").ap()
    d["dwm"] = nc.dram_tensor("dwm", [8, 128, 16 * 2 * 128], F8,
                              kind="ExternalInput").ap()
    d["dw_b"] = nc.dram_tensor("dw_b", [CONV_IN], F32, kind="ExternalInput").ap()
    d["pw2_t"] = nc.dram_tensor("pw2_t", [CONV_IN, D], F8, kind="ExternalInput").ap()
    d["pw2_b"] = nc.dram_tensor("pw2_b", [D], F32, kind="ExternalInput").ap()
    d["fn_g"] = nc.dram_tensor("fn_g", [D], F32, kind="ExternalInput").ap()
    d["fn_b"] = nc.dram_tensor("fn_b", [D], F32, kind="ExternalInput").ap()
    d["out"] = nc.dram_tensor("out", [D, S], F32, kind="ExternalOutput").ap()
    if debug:
        for i in range(1, 5):
            d[f"dbg{i}"] = nc.dram_tensor(f"dbg{i}", [D, S], F32,
                                          kind="ExternalOutput").ap()
        d["dbgh"] = nc.dram_tensor("dbgh", [D, S], F32, kind="ExternalOutput").ap()

    from contextlib import ExitStack
    with tile.TileContext(nc) as tc, ExitStack() as ctx:
        cpool = ctx.enter_context(tc.tile_pool(name="cpool", bufs=1))
        spool = ctx.enter_context(tc.tile_pool(name="spool", bufs=1))
        bp = ctx.enter_context(tc.tile_pool(name="bp", bufs=1))
        ps_s = ctx.enter_context(tc.tile_pool(name="ps_s", bufs=1, space="PSUM"))

        ones = cpool.tile([128, 128], F32R)
        nc.sync.dma_start(ones[:], d["ones"])

        consts = cpool.tile([128, 2], F32, tag="consts")
        nc.gpsimd.memset(consts[:, 0:1], LN_EPS)
        nc.gpsimd.memset(consts[:, 1:2], float(np.log(EU)))
        eps_ap = consts[:, 0:1]
        lneu_ap = consts[:, 1:2]

        xs = spool.tile([128, FC, S], F32R)
        nc.sync.dma_start(xs[:], d["x"].rearrange("(c p) n -> p c n", p=128))

        # per-partition bias rows [128, nchunks] (only loaded when nonzero)
        brow = {}
        for name, width in (("ff1_bin", 2 * FF_HID), ("ff1_bout", D),
                            ("ff2_bin", 2 * FF_HID), ("ff2_bout", D),
                            ("bqkv", 3 * D), ("bo", D), ("pw1_b", 2 * CONV_IN),
                            ("dw_b", CONV_IN), ("pw2_b", D)):
            if bias_nz.get(name):
                t = bp.tile([128, width // 128], F32, tag=name)
                nc.sync.dma_start(t[:], d[name].rearrange("(c p) -> p c", p=128))
                brow[name] = t

        def bias_ap(name, chunk):
            return brow[name][:, chunk:chunk + 1]

        # ------------------------------------------------------------------
        def layer_norm(lnp, h8=None, hf=None):
            """LN over both token chunks, software-pipelined: stats matmuls
            for t0/t1 interleaved, fused squares/apply ops with stride-0
            broadcast of the per-token stats."""
            bcs = []
            for t in range(TC):
                sl = slice(t * TN, (t + 1) * TN)
                bc_s = ps_s.tile([128, TN], F32, tag="s", name="bc_s")
                for c in range(FC):
                    nc.tensor.matmul(bc_s[:], ones[:], xs[:, c, sl],
                                     start=(c == 0), stop=(c == FC - 1))
                bcs.append(bc_s)
            stats = []
            for t in range(TC):
                sl = slice(t * TN, (t + 1) * TN)
                xsq = lnp.tile([128, FC, TN], F32R, tag="xsq")
                nc.gpsimd.tensor_tensor(xsq[:], xs[:, :, sl].bitcast(F32),
                                        xs[:, :, sl].bitcast(F32), ALU.mult)
                bc_q = ps_s.tile([128, TN], F32, tag="q", name="bc_q")
                for c in range(FC):
                    nc.tensor.matmul(bc_q[:], ones[:], xsq[:, c, :],
                                     start=(c == 0), stop=(c == FC - 1))
                mu = lnp.tile([128, TN], F32, tag="mu")
                nc.vector.tensor_scalar(mu[:], bcs[t][:], 1.0 / D, None, ALU.mult)
                m2 = lnp.tile([128, TN], F32, tag="m2")
                nc.gpsimd.tensor_tensor(m2[:], mu[:], mu[:], ALU.mult)
                t1 = lnp.tile([128, TN], F32, tag="t1")
                nc.vector.scalar_tensor_tensor(t1[:], m2[:], -float(D), bc_q[:],
                                               ALU.mult, ALU.add)
                lnt = lnp.tile([128, TN], F32, tag="lnt")
                nc.scalar.activation(lnt[:], t1[:], AF.Ln, scale=1.0 / D,
                                     bias=eps_ap)
                rsig = lnp.tile([128, TN], F32, tag="rsig")
                nc.scalar.activation(rsig[:], lnt[:], AF.Exp, scale=-0.5)
                bt = lnp.tile([128, TN], F32, tag="bt")
                nc.gpsimd.tensor_tensor(bt[:], mu[:], rsig[:], ALU.mult)
                stats.append((rsig, bt))
            for t in range(TC):
                sl = slice(t * TN, (t + 1) * TN)
                rsig, bt = stats[t]
                rb = rsig[:, None, :].broadcast_to([128, FC, TN])
                bb = bt[:, None, :].broadcast_to([128, FC, TN])
                xr = lnp.tile([128, FC, TN], F32, tag="xr")
                nc.gpsimd.tensor_tensor(xr[:], xs[:, :, sl].bitcast(F32),
                                        rb, ALU.mult)
                dst = h8[:, :, sl] if h8 is not None else hf[:, :, sl]
                nc.vector.tensor_tensor(dst, xr[:], bb, ALU.subtract)

        # ------------------------------------------------------------------
        def ffn(tag, pools, dbg=False):
            lnp, hpool, wff, fsp, ps_f = pools
            h = hpool.tile([128, FC, S], F8, tag="h")
            layer_norm(lnp, h8=h)
            if dbg:
                hdbg = hpool.tile([128, FC, S], F32, tag="hdbg")
                for c in range(FC):
                    nc.vector.tensor_scalar(hdbg[:, c, :], h[:, c, :], 1.0,
                                            None, ALU.mult)
                nc.sync.dma_start(d["dbgh"].rearrange("(c p) n -> p c n", p=128),
                                  hdbg[:])
            w_in = wff.tile([128, FC, 2 * FF_HID], F8, tag="wffin")
            nc.sync.dma_start(w_in[:],
                              d[f"{tag}_win_t"].rearrange("(c p) m -> p c m", p=128))
            nh = FF_HID // 128  # 16
            w_out = wff.tile([128, nh, D], F8, tag="wffout")
            nc.sync.dma_start(w_out[:],
                              d[f"{tag}_wout_t"].rearrange("(c p) m -> p c m", p=128))
            b_in = f"{tag}_bin"
            b_out = f"{tag}_bout"
            s_in = 1.0 / sc[f"{tag}_win"]
            s_g = GU / sc[f"{tag}_win"]
            s_out = 1.0 / (GU * sc[f"{tag}_wout"])
            for t in range(TC):
                sl = slice(t * TN, (t + 1) * TN)
                accs = [ps_f.tile([128, TN], F32, tag="acc", name="acc")
                        for _ in range(FC)]
                for hp in range(nh // 2):        # hc pairs
                    g2 = fsp.tile([128, 2, TN], F8, tag="g2")
                    for sub in range(2):
                        hc = 2 * hp + sub
                        a_ps = ps_f.tile([128, TN], F32, tag="s", name="a_ps", bufs=2)
                        nc.tensor.matmul(a_ps[:], w_in[:, 0:2, hc * 128:(hc + 1) * 128],
                                         h[:, 0:2, sl], start=True, stop=False,
                                         perf_mode=DR)
                        nc.tensor.matmul(a_ps[:], w_in[:, 2:4, hc * 128:(hc + 1) * 128],
                                         h[:, 2:4, sl], start=False, stop=True,
                                         perf_mode=DR)
                        mo = nh + hc
                        c_ps = ps_f.tile([128, TN], F32, tag="s", name="c_ps", bufs=2)
                        nc.tensor.matmul(c_ps[:], w_in[:, 0:2, mo * 128:(mo + 1) * 128],
                                         h[:, 0:2, sl], start=True, stop=False,
                                         perf_mode=DR)
                        nc.tensor.matmul(c_ps[:], w_in[:, 2:4, mo * 128:(mo + 1) * 128],
                                         h[:, 2:4, sl], start=False, stop=True,
                                         perf_mode=DR)
                        a_sb = fsp.tile([128, TN], F32, tag="asb")
                        nc.scalar.activation(
                            a_sb[:], a_ps[:], AF.Silu, scale=s_in,
                            bias=bias_ap(b_in, hc) if bias_nz[b_in] else 0.0)
                        if bias_nz[b_in]:
                            csb = fsp.tile([128, TN], F32, tag="csb")
                            nc.scalar.activation(csb[:], c_ps[:], AF.Identity,
                                                 scale=s_g, bias=bias_ap(b_in, mo))
                            nc.vector.tensor_tensor(g2[:, sub, :], csb[:],
                                                    a_sb[:], ALU.mult)
                        else:
                            nc.vector.scalar_tensor_tensor(
                                g2[:, sub, :], c_ps[:], s_g, a_sb[:],
                                ALU.mult, ALU.mult)
                    for m in range(FC):
                        nc.tensor.matmul(accs[m][:],
                                         w_out[:, 2 * hp:2 * hp + 2,
                                               m * 128:(m + 1) * 128],
                                         g2[:, 0:2, :],
                                         start=(hp == 0), stop=(hp == nh // 2 - 1),
                                         perf_mode=DR)
                for m in range(FC):
                    if bias_nz[b_out]:
                        asb = fsp.tile([128, TN], F32, tag="rsb")
                        nc.scalar.activation(asb[:], accs[m][:], AF.Identity,
                                             scale=s_out, bias=bias_ap(b_out, m))
                        nc.vector.tensor_tensor(xs[:, m, sl], asb[:],
                                                xs[:, m, sl].bitcast(F32), ALU.add)
                    else:
                        nc.vector.scalar_tensor_tensor(
                            xs[:, m, sl], accs[m][:], s_out,
                            xs[:, m, sl].bitcast(F32), ALU.mult, ALU.add)

        # ------------------------------------------------------------------
        def attention(pools):
            lnp, hpool, watt, attp, att2, att3 = pools
            h = hpool.tile([128, FC, S], F8, tag="h")
            layer_norm(lnp, h8=h)
            wqkv = watt.tile([128, FC, 3 * D], F8, tag="wqkv")
            nc.sync.dma_start(wqkv[:],
                              d["wqkv_t"].rearrange("(c p) m -> p c m", p=128))
            wo = watt.tile([128, FC, D], F8, tag="wo")
            nc.sync.dma_start(wo[:], d["wo_t"].rearrange("(c p) m -> p c m", p=128))
            s_qkv = 1.0 / sc["wqkv"]
            bvq = None
            if bias_nz["bqkv"]:
                bvq = bp.tile([128, FC], F32, tag="bvq")
                nc.sync.dma_start(bvq[:],
                                  d["bqkv"][2 * D:3 * D].rearrange("(c p) -> p c",
                                                                   p=128))
                # o_fm carries an OU upscale; pre-scale the v bias to match
                nc.vector.tensor_scalar(bvq[:], bvq[:], OU, None, ALU.mult)

            q_sb = attp.tile([128, FC, S], BF16, tag="q")
            k_sb = attp.tile([128, FC, S], BF16, tag="k")
            vaug = attp.tile([128, 8, HEADS, 66], F8, tag="vaug")
            nc.gpsimd.memset(vaug[:, :, :, 64:65], 1.0)
            with ExitStack() as ist:
                ps_qkv = ist.enter_context(
                    tc.tile_pool(name="ps_qkv", bufs=4, space="PSUM"))
                for fc in range(FC):
                    for t in range(TC):
                        sl = slice(t * TN, (t + 1) * TN)
                        for which, base in (("q", 0), ("k", D)):
                            pp = ps_qkv.tile([128, TN], F32, tag="s", name="pp")
                            mo = base // 128 + fc
                            nc.tensor.matmul(pp[:],
                                             wqkv[:, 0:2, mo * 128:(mo + 1) * 128],
                                             h[:, 0:2, sl], start=True, stop=False,
                                             perf_mode=DR)
                            nc.tensor.matmul(pp[:],
                                             wqkv[:, 2:4, mo * 128:(mo + 1) * 128],
                                             h[:, 2:4, sl], start=False, stop=True,
                                             perf_mode=DR)
                            dst = q_sb if which == "q" else k_sb
                            nc.vector.tensor_scalar(dst[:, fc, sl], pp[:],
                                                    s_qkv, None, ALU.mult)
                            if bias_nz["bqkv"]:
                                nc.vector.tensor_scalar(
                                    dst[:, fc, sl], dst[:, fc, sl].bitcast(BF16),
                                    bias_ap("bqkv", mo), None, ALU.add)

                # v token-major with fp8 ones column at index 64 per head
                for kc in range(8):
                    v_ps = ps_qkv.tile([128, D], F32, tag="s", name="v_ps")
                    nc.tensor.matmul(v_ps[:], h[:, 0:2, kc * 128:(kc + 1) * 128],
                                     wqkv[:, 0:2, 2 * D:3 * D], start=True,
                                     stop=False, perf_mode=DR)
                    nc.tensor.matmul(v_ps[:], h[:, 2:4, kc * 128:(kc + 1) * 128],
                                     wqkv[:, 2:4, 2 * D:3 * D], start=False,
                                     stop=True, perf_mode=DR)
                    nc.vector.tensor_scalar(
                        vaug[:, kc, :, 0:64],
                        v_ps[:].rearrange("p (h e) -> p h e", h=HEADS),
                        s_qkv, None, ALU.mult)

            o_fm = attp.tile([128, FC, S], F8, tag="ofm")
            sm_scale = float(DH) ** -0.5
            with ExitStack() as ist:
                ps_sc = ist.enter_context(
                    tc.tile_pool(name="ps_sc", bufs=1, space="PSUM"))
                ps_o = ist.enter_context(
                    tc.tile_pool(name="ps_o", bufs=1, space="PSUM"))
                ps_bc = ist.enter_context(
                    tc.tile_pool(name="ps_bc", bufs=1, space="PSUM"))
                for hd in range(HEADS):
                    hb = (hd % 2) * 64
                    hc = hd // 2
                    for t in range(TC):
                        sl = slice(t * TN, (t + 1) * TN)
                        e_sb = att2.tile([128, 8, TN], F8, tag="esb")
                        o_ps = ps_o.tile([65, TN], F32, tag="acc")
                        s4 = ps_sc.tile([128, 4, TN], F32, tag="s4")
                        for kc in range(8):
                            nc.tensor.matmul(
                                s4[:, kc % 4, :],
                                k_sb[hb:hb + 64, hc, kc * 128:(kc + 1) * 128],
                                q_sb[hb:hb + 64, hc, sl],
                                start=True, stop=True)
                            if kc % 4 == 3:
                                q4 = kc // 4
                                nc.scalar.activation(
                                    e_sb[:, 4 * q4:4 * q4 + 4, :], s4[:],
                                    AF.Exp, scale=sm_scale, bias=lneu_ap)
                        for kp in range(4):
                            nc.tensor.matmul(o_ps[:],
                                             vaug[:, 2 * kp:2 * kp + 2, hd, 0:65],
                                             e_sb[:, 2 * kp:2 * kp + 2, :],
                                             start=(kp == 0), stop=(kp == 3),
                                             perf_mode=DR)
                        rows = att3.tile([1, 3, TN], F32, tag="rows")
                        nc.scalar.activation(rows[:, 2, :], o_ps[64:65, :], AF.Copy)
                        nc.vector.reciprocal_approx_accurate(
                            rows[:, 0, :], rows[:, 2, :], rows[:, 1, :])
                        rrow_r = att3.tile([1, TN], F32R, tag="rrowr")
                        nc.scalar.activation(rrow_r[:], rows[:, 0, :], AF.Copy,
                                             scale=OU)
                        bc_ps = ps_bc.tile([64, TN], F32, tag="bc")
                        nc.tensor.matmul(bc_ps[:], ones[0:1, 0:64], rrow_r[:],
                                         start=True, stop=True)
                        bc_sb = att3.tile([64, TN], F32, tag="bcsb")
                        nc.scalar.activation(bc_sb[:], bc_ps[:], AF.Copy)
                        nc.vector.tensor_tensor(o_fm[hb:hb + 64, hc, sl],
                                                o_ps[0:64, :], bc_sb[:], ALU.mult)
                        if bias_nz["bqkv"]:
                            # + v bias (softmax weights sum to one; bvq pre-scaled)
                            nc.vector.tensor_scalar(
                                o_fm[hb:hb + 64, hc, sl],
                                o_fm[hb:hb + 64, hc, sl].bitcast(F8),
                                bvq[hb:hb + 64, hc:hc + 1], None, ALU.add)

            s_o = 1.0 / (OU * sc["wo"])
            with ExitStack() as ist:
                ps_pr = ist.enter_context(
                    tc.tile_pool(name="ps_pr", bufs=2, space="PSUM"))
                for t in range(TC):
                    sl = slice(t * TN, (t + 1) * TN)
                    for m in range(FC):
                        acc = ps_pr.tile([128, TN], F32, tag="acc")
                        nc.tensor.matmul(acc[:], wo[:, 0:2, m * 128:(m + 1) * 128],
                                         o_fm[:, 0:2, sl], start=True, stop=False,
                                         perf_mode=DR)
                        nc.tensor.matmul(acc[:], wo[:, 2:4, m * 128:(m + 1) * 128],
                                         o_fm[:, 2:4, sl], start=False, stop=True,
                                         perf_mode=DR)
                        if bias_nz["bo"]:
                            tmp = att3.tile([128, TN], F32, tag="tmp")
                            nc.scalar.activation(tmp[:], acc[:], AF.Identity,
                                                 scale=s_o, bias=bias_ap("bo", m))
                            nc.vector.tensor_tensor(xs[:, m, sl], tmp[:],
                                                    xs[:, m, sl].bitcast(F32),
                                                    ALU.add)
                        else:
                            nc.vector.scalar_tensor_tensor(
                                xs[:, m, sl], acc[:], s_o,
                                xs[:, m, sl].bitcast(F32), ALU.mult, ALU.add)

        # ------------------------------------------------------------------
        def conv(pools, ntaps=16):
            lnp, hpool, wconv, wdw, convp, up, fsp = pools
            h = hpool.tile([128, FC, S], F8, tag="h")
            layer_norm(lnp, h8=h)
            pw1 = wconv.tile([128, FC, 2 * CONV_IN], F8, tag="pw1")
            nc.sync.dma_start(pw1[:],
                              d["pw1_t"].rearrange("(c p) m -> p c m", p=128))
            ncc = CONV_IN // 128  # 8
            pw2 = wconv.tile([128, ncc, D], F8, tag="pw2")
            nc.sync.dma_start(pw2[:],
                              d["pw2_t"].rearrange("(c p) m -> p c m", p=128))
            s_p1 = 1.0 / sc["pw1"]
            s_u = UU / sc["pw1"]
            s_dw = 1.0 / (UU * sc["dw"])
            s_p2 = 1.0 / sc["pw2"]

            dvo = convp.tile([128, ncc, S], F8, tag="dvo")
            u2s = []
            # ---- pass A: pw1 + GLU for all channel chunks (PE runs ahead,
            # Act/DVE/Pool trail; u2 tiles all stay resident) ----
            with ExitStack() as ist:
                ps_ac = ist.enter_context(
                    tc.tile_pool(name="ps_ac", bufs=2, space="PSUM"))
                for cc in range(ncc):
                    u2 = up.tile([128, 2, UL], F8, tag=f"u{cc}", name="u2")
                    u2s.append(u2)
                    nc.gpsimd.memset(u2[:, :, 0:PAD], 0.0)
                    nc.gpsimd.memset(u2[:, :, PAD + S:], 0.0)
                    mo = ncc + cc
                    for t in range(TC):
                        sl = slice(t * TN, (t + 1) * TN)
                        ac = ps_ac.tile([128, 2, TN], F32, tag="ac")
                        for cp in range(2):
                            nc.tensor.matmul(
                                ac[:, 0, :],
                                pw1[:, 2 * cp:2 * cp + 2, cc * 128:(cc + 1) * 128],
                                h[:, 2 * cp:2 * cp + 2, sl],
                                start=(cp == 0), stop=(cp == 1), perf_mode=DR)
                        for cp in range(2):
                            nc.tensor.matmul(
                                ac[:, 1, :],
                                pw1[:, 2 * cp:2 * cp + 2, mo * 128:(mo + 1) * 128],
                                h[:, 2 * cp:2 * cp + 2, sl],
                                start=(cp == 0), stop=(cp == 1), perf_mode=DR)
                        sg = fsp.tile([128, TN], F32, tag="sg")
                        nc.scalar.activation(
                            sg[:], ac[:, 1, :], AF.Sigmoid, scale=s_p1,
                            bias=bias_ap("pw1_b", mo) if bias_nz["pw1_b"] else 0.0)
                        if bias_nz["pw1_b"]:
                            asb = fsp.tile([128, TN], F32, tag="asb")
                            nc.scalar.activation(asb[:], ac[:, 0, :], AF.Identity,
                                                 scale=s_u, bias=bias_ap("pw1_b", cc))
                            nc.vector.tensor_tensor(
                                u2[:, 0, PAD + t * TN:PAD + (t + 1) * TN],
                                asb[:], sg[:], ALU.mult)
                        else:
                            nc.vector.scalar_tensor_tensor(
                                u2[:, 0, PAD + t * TN:PAD + (t + 1) * TN],
                                ac[:, 0, :], s_u, sg[:], ALU.mult, ALU.mult)
                    # shifted copy for odd taps: u2[:,1,i] = u2[:,0,i+1]
                    nc.gpsimd.tensor_scalar(u2[:, 1, PAD - 1:PAD + S],
                                            u2[:, 0, PAD:PAD + S + 1], 1.0, None,
                                            ALU.mult)
            # ---- pass B: depthwise tap-pair matmuls, both chunks into one
            # 2-bank psum, one fused silu per chunk pair ----
            with ExitStack() as ist:
                ps_dw = ist.enter_context(
                    tc.tile_pool(name="ps_dw", bufs=2, space="PSUM"))
                for cc in range(ncc):
                    dwW = wdw.tile([128, 16, 2, 128], F8, tag="dww")
                    nc.sync.dma_start(
                        dwW[:], d["dwm"][cc].rearrange("p (t s m) -> p t s m",
                                                       t=16, s=2))
                    acc2 = ps_dw.tile([128, 2, TN], F32, tag="dw")
                    for t in range(TC):
                        for pr in range(ntaps):
                            nc.tensor.matmul(acc2[:, t, :], dwW[:, pr, :, :],
                                             u2s[cc][:, 0:2, t * TN + 2 * pr:
                                                     t * TN + 2 * pr + TN],
                                             start=(pr == 0),
                                             stop=(pr == ntaps - 1),
                                             perf_mode=DR)
                    nc.scalar.activation(
                        dvo[:, cc, :].rearrange("p (t n) -> p t n", t=TC),
                        acc2[:], AF.Silu, scale=s_dw,
                        bias=bias_ap("dw_b", cc) if bias_nz["dw_b"] else 0.0)

            # ---- pass C: pw2 + residual ----
            with ExitStack() as ist:
                ps_o = ist.enter_context(
                    tc.tile_pool(name="ps_o", bufs=2, space="PSUM"))
                for t in range(TC):
                    sl = slice(t * TN, (t + 1) * TN)
                    for m in range(FC):
                        acc = ps_o.tile([128, TN], F32, tag="acc")
                        for cp in range(4):
                            nc.tensor.matmul(acc[:],
                                             pw2[:, 2 * cp:2 * cp + 2,
                                                 m * 128:(m + 1) * 128],
                                             dvo[:, 2 * cp:2 * cp + 2, sl],
                                             start=(cp == 0), stop=(cp == 3),
                                             perf_mode=DR)
                        if bias_nz["pw2_b"]:
                            tmp = fsp.tile([128, TN], F32, tag="tmp")
                            nc.scalar.activation(tmp[:], acc[:], AF.Identity,
                                                 scale=s_p2, bias=bias_ap("pw2_b", m))
                            nc.vector.tensor_tensor(xs[:, m, sl], tmp[:],
                                                    xs[:, m, sl].bitcast(F32),
                                                    ALU.add)
                        else:
                            nc.vector.scalar_tensor_tensor(
                                xs[:, m, sl], acc[:], s_p2,
                                xs[:, m, sl].bitcast(F32), ALU.mult, ALU.add)

        # ------------------------------------------------------------------
        def ff_pools(st):
            return (st.enter_context(tc.tile_pool(name="lnp", bufs=2)),
                    st.enter_context(tc.tile_pool(name="hp", bufs=1)),
                    st.enter_context(tc.tile_pool(name="wff", bufs=1)),
                    st.enter_context(tc.tile_pool(name="fsp", bufs=3)),
                    st.enter_context(tc.tile_pool(name="ps_f", bufs=4,
                                                  space="PSUM")))

        def dbg_tap(i):
            if debug:
                nc.sync.dma_start(d[f"dbg{i}"].rearrange("(c p) n -> p c n", p=128),
                                  xs[:].bitcast(F32))

        for _rep in range(nreps):
            dbg = debug and _rep == nreps - 1
            if "ff1" in phases:
                with ExitStack() as st:
                    ffn("ff1", ff_pools(st), dbg=dbg)
            if dbg:
                dbg_tap(1)
            if "attn" in phases:
                with ExitStack() as st:
                    pools = (st.enter_context(tc.tile_pool(name="lnp", bufs=2)),
                             st.enter_context(tc.tile_pool(name="hp", bufs=1)),
                             st.enter_context(tc.tile_pool(name="watt", bufs=1)),
                             st.enter_context(tc.tile_pool(name="attp", bufs=1)),
                             st.enter_context(tc.tile_pool(name="att2", bufs=2)),
                             st.enter_context(tc.tile_pool(name="att3", bufs=2)))
                    attention(pools)
            if dbg:
                dbg_tap(2)
            if "convt2" in phases:
                with ExitStack() as st:
                    pools = (st.enter_context(tc.tile_pool(name="lnp", bufs=2)),
                             st.enter_context(tc.tile_pool(name="hp", bufs=1)),
                             st.enter_context(tc.tile_pool(name="wconv", bufs=1)),
                             st.enter_context(tc.tile_pool(name="wdw", bufs=2)),
                             st.enter_context(tc.tile_pool(name="convp", bufs=1)),
                             st.enter_context(tc.tile_pool(name="up", bufs=3)),
                             st.enter_context(tc.tile_pool(name="fsp", bufs=2)))
                    conv(pools, ntaps=2)
            if "conv" in phases:
                with ExitStack() as st:
                    pools = (st.enter_context(tc.tile_pool(name="lnp", bufs=2)),
                             st.enter_context(tc.tile_pool(name="hp", bufs=1)),
                             st.enter_context(tc.tile_pool(name="wconv", bufs=1)),
                             st.enter_context(tc.tile_pool(name="wdw", bufs=2)),
                             st.enter_context(tc.tile_pool(name="convp", bufs=1)),
                             st.enter_context(tc.tile_pool(name="up", bufs=3)),
                             st.enter_context(tc.tile_pool(name="fsp", bufs=2)))
                    conv(pools)
            if dbg:
                dbg_tap(3)
            if "ff2" in phases:
                with ExitStack() as st:
                    ffn("ff2", ff_pools(st))
            if dbg:
                dbg_tap(4)

        with ExitStack() as st:
            lnp = st.enter_context(tc.tile_pool(name="lnp", bufs=2))
            outt = spool.tile([128, FC, S], F32, tag="outt")
            layer_norm(lnp, hf=outt)
            if bias_nz["fn"]:
                fg = cpool.tile([128, FC], F32, tag="fg")
                nc.sync.dma_start(fg[:], d["fn_g"].rearrange("(c p) -> p c", p=128))
                fb = cpool.tile([128, FC], F32, tag="fb")
                nc.sync.dma_start(fb[:], d["fn_b"].rearrange("(c p) -> p c", p=128))
                for c in range(FC):
                    nc.vector.tensor_scalar(outt[:, c, :], outt[:, c, :],
                                            fg[:, c:c + 1], fb[:, c:c + 1],
                                            ALU.mult, ALU.add)
        nc.sync.dma_start(d["out"].rearrange("(c p) n -> p c n", p=128), outt[:])

    nc.compile()
    return nc


# --------------------------------------------------------------------------
# SPMD execution (replicates bass2jax.run_bass_via_pjrt, reusable executable)
# --------------------------------------------------------------------------

class _Runner:
    def __init__(self, nc, n_cores=8):
        import jax
        from jax.sharding import Mesh, PartitionSpec
        from jax.experimental.shard_map import shard_map
        from concourse.bass2jax import (
            _bass_exec_p, install_neuronx_cc_hook, partition_id_tensor,
        )
        install_neuronx_cc_hook()
        self.jax = jax
        self.n_cores = n_cores
        partition_name = (nc.partition_id_tensor.name
                          if nc.partition_id_tensor else None)
        in_names, out_names, out_avals, zero_outs = [], [], [], []
        for alloc in nc.m.functions[0].allocations:
            if not isinstance(alloc, mybir.MemoryLocationSet):
                continue
            name = alloc.memorylocations[0].name
            if alloc.kind == "ExternalInput":
                if name != partition_name:
                    in_names.append(name)
            elif alloc.kind == "ExternalOutput":
                shape = tuple(alloc.tensor_shape)
                dtype = mybir.dt.np(alloc.dtype)
                out_names.append(name)
                out_avals.append(jax.core.ShapedArray(shape, dtype))
                zero_outs.append(np.zeros(shape, dtype))
        self.in_names, self.out_names = in_names, out_names
        self.out_avals, self.zero_outs = out_avals, zero_outs
        n_params, n_outs = len(in_names), len(out_avals)
        all_in = list(in_names) + list(out_names)
        if partition_name is not None:
            all_in.append(partition_name)
        donate = tuple(range(n_params, n_params + n_outs))

        def _body(*args):
            operands = list(args)
            if partition_name is not None:
                operands.append(partition_id_tensor())
            return tuple(_bass_exec_p.bind(
                *operands, out_avals=tuple(out_avals), in_names=tuple(all_in),
                out_names=tuple(out_names), lowering_input_output_aliases=(),
                sim_require_finite=True, sim_require_nnan=True, nc=nc))

        devices = jax.devices()[:n_cores]
        mesh = Mesh(np.asarray(devices), ("core",))
        in_specs = (PartitionSpec("core"),) * (n_params + n_outs)
        out_specs = (PartitionSpec("core"),) * n_outs
        self._fn = jax.jit(
            shard_map(_body, mesh=mesh, in_specs=in_specs, out_specs=out_specs,
                      check_rep=False),
            donate_argnums=donate, keep_unused=True)

    def concat_inputs(self, in_maps):
        n = self.n_cores
        per_core = [[np.asarray(m[name]) for name in self.in_names]
                    for m in in_maps]
        return [np.concatenate([per_core[c][i] for c in range(n)], axis=0)
                for i in range(len(self.in_names))]

    def run_concat(self, concat_in):
        n = self.n_cores
        zeros = [np.zeros((n * z.shape[0], *z.shape[1:]), z.dtype)
                 for z in self.zero_outs]
        out = self._fn(*concat_in, *zeros)
        self.jax.block_until_ready(out)
        return out

    def __call__(self, in_maps):
        out = self.run_concat(self.concat_inputs(in_maps))
        n = self.n_cores
        return [
            {name: np.asarray(out[i]).reshape(n, *self.out_avals[i].shape)[c]
             for i, name in enumerate(self.out_names)}
            for c in range(n)
        ]


def _freeze(obj):
    if isinstance(obj, dict):
        return tuple(sorted((k, _freeze(v)) for k, v in obj.items()))
    return obj


def _get_runner(cfg, debug=False, nreps=1,
                phases=("ff1", "attn", "conv", "ff2")):
    key = (_freeze(cfg), debug, nreps, tuple(phases))
    if key not in _CACHE:
        _CACHE[key] = _Runner(
            _build(cfg, debug=debug, nreps=nreps, phases=phases), 8)
    return _CACHE[key]


def _make_in_maps(inputs):
    p, scales = _prep(inputs)
    x = np.asarray(inputs["x"], np.float32)
    bias_nz = {
        "ff1_bin": bool(np.any(p["ff1_bin"])), "ff1_bout": bool(np.any(p["ff1_bout"])),
        "ff2_bin": bool(np.any(p["ff2_bin"])), "ff2_bout": bool(np.any(p["ff2_bout"])),
        "bqkv": bool(np.any(p["bqkv"])), "bo": bool(np.any(p["bo"])),
        "pw1_b": bool(np.any(p["pw1_b"])), "dw_b": bool(np.any(p["dw_b"])),
        "pw2_b": bool(np.any(p["pw2_b"])),
        "fn": bool(np.any(p["fn_g"] != 1.0) or np.any(p["fn_b"])),
    }
    cfg = {"scales": scales, "bias_nz": bias_nz}
    p["dwm"] = p["dwm"].reshape(8, 128, 16 * 2 * 128)
    shared = {k: p[k] for k in
              ("ones", "ff1_win_t", "ff1_wout_t", "ff1_bin", "ff1_bout",
               "ff2_win_t", "ff2_wout_t", "ff2_bin", "ff2_bout",
               "wqkv_t", "bqkv", "wo_t", "bo", "pw1_t", "pw1_b", "dwm", "dw_b",
               "pw2_t", "pw2_b", "fn_g", "fn_b")}
    in_maps = []
    for b in range(B):
        m = dict(shared)
        m["x"] = np.ascontiguousarray(x[b].T)          # [512, 1024]
        in_maps.append(m)
    return in_maps, cfg


def kernel(**inputs):
    in_maps, cfg = _make_in_maps(inputs)
    runner = _get_runner(cfg)
    results = runner(in_maps)
    out = np.stack([results[b]["out"].T for b in range(B)], axis=0)
    return np.ascontiguousarray(out.astype(np.float32))
